# revision 19
# baseline (speedup 1.0000x reference)
"""GNN message-passing (NodeModel) Trainium2 kernel.

Computation (per reference):
    h   = relu(relu(concat(x[row], ea) @ W0 + b0) @ W1 + b1) @ W2 + b2   [E, 128]
    agg = segment_sum(h, col, N)                                          [N, 128]
    out = relu(relu(concat(x, agg) @ V0 + c0) @ V1 + c1) @ V2 + c2       [N, 128]

Distribution: edges are sorted by destination node; each of the 8 cores owns a
contiguous range of 6250 destination nodes and all edges pointing into it, so
no cross-core reduction is needed.  The host pre-gathers x[row] (and
transposes operands) into tile-major streaming layouts; all matmuls run in
bf16 with fp32 PSUM accumulation.

Device pipeline per core, processed in PAIRS of 512-edge groups (1024 edge
slots per pair) so L1 runs as N=1024 matmuls (one LDWEIGHTS per two groups
and 2 KB DMA descriptor rows):
  L1: h1^T = relu(W0x^T xrow^T + W0e^T ea^T + b0)      (ACT bias+relu)
  L2: h2   = relu(h1^T.T @ W1 + b1)  -- "swap" matmul per 512-group
      producing h2 in natural [edge, feat] orientation.
  Aggregation over W2 pushed past the segment sum:
      u^T[:, n] = sum_{e in seg(n)} h2[e, :]  via matmul with an on-chip
      generated one-hot segment matrix (DVE is_equal against an iota const),
      accumulated per 128-node window in PSUM.
  MLP2 with W2/V0a fused through the linear aggregation, INTERLEAVED into
  phase A as each 512-node chunk's windows complete:
      g1^T = relu(V0x^T x^T + (W2 V0a)^T u^T + (V0a^T b2) (x) deg + c0)
      out^T = V2^T relu(V1^T g1^T + c1) + c2, stored bf16.
"""

import os
import numpy as np
import ml_dtypes

import concourse.bass as bass
import concourse.bacc as bacc
import concourse.mybir as mybir
import concourse.tile as tile
from concourse.bass_utils import run_bass_kernel_spmd

BF16 = ml_dtypes.bfloat16

N_NODES = 50000
N_EDGES = 800000
NODE_F = 128
EDGE_F = 64
HID = 128
NCORES = 8
NPC = N_NODES // NCORES  # 6250 nodes per core
WIN = 128                # nodes per aggregation window (PSUM columns)


def _f32(a):
    return np.ascontiguousarray(a, dtype=np.float32)


def _bf(a):
    return np.ascontiguousarray(a, dtype=BF16)


# ---------------------------------------------------------------------------
# Host-side packing
# ---------------------------------------------------------------------------

def _plan_windows(deg_core, cap_edges, max_nodes=WIN):
    """Greedy node->window packing: consecutive nodes per window, capped at
    max_nodes nodes and cap_edges edges. Returns list of (start, n_nodes)."""
    wins = []
    s, n = 0, len(deg_core)
    while s < n:
        e = 0
        c = 0
        while s + c < n and c < max_nodes and e + deg_core[s + c] <= cap_edges:
            e += deg_core[s + c]
            c += 1
        if c == 0:
            c = 1
        wins.append((s, c))
        s += c
    return wins


def _pack_core(rows, cols, ea_sorted_bf, x_bf, node_lo, wins, nw, tpw):
    """Build per-core device input arrays.

    rows/cols: this core's edges sorted by col.
    wins: list of (node_start, n_nodes) windows for this core (len <= nw).
    Returns input dict + col->global-node map for output reassembly.
    """
    t_tiles = nw * tpw
    t4 = -(-t_tiles // 8) * 8
    epad = t4 * 128
    npair = t4 // 8
    nodes_pad = nw * WIN
    npc_k = max(w[0] + w[1] for w in wins)

    win_of_node = np.zeros(npc_k, dtype=np.int64)
    start_of_node = np.zeros(npc_k, dtype=np.int64)
    for w, (s, c) in enumerate(wins):
        win_of_node[s:s + c] = w
        start_of_node[s:s + c] = s

    local_node = cols - node_lo
    win = win_of_node[local_node]
    win_first = np.searchsorted(win, np.arange(nw))
    j = np.arange(len(cols)) - win_first[win]
    slot = win * (tpw * 128) + j
    assert j.max(initial=0) < tpw * 128

    xrowT = np.zeros((NODE_F, epad), dtype=BF16)
    xrowT[:, slot] = x_bf[rows].T
    # pair-major: each 1024-slot pair is one contiguous [128, 1024] block
    xrT = np.ascontiguousarray(
        xrowT.reshape(NODE_F, npair // 2, 2048).transpose(1, 0, 2))

    ea_full = np.zeros((EDGE_F, epad), dtype=BF16)
    ea_full[:, slot] = ea_sorted_bf.T
    eaT = np.ascontiguousarray(
        ea_full.reshape(EDGE_F, npair // 2, 2048).transpose(1, 0, 2))

    colloc = np.full((128, t4), -1.0, dtype=BF16)
    local = local_node - start_of_node[local_node]
    colloc[slot % 128, slot // 128] = local.astype(BF16)

    # column j = w*WIN + c  ->  global node node_lo + start_w + c  (c < c_w)
    col2node = np.full(nodes_pad, -1, dtype=np.int64)
    for w, (s, c) in enumerate(wins):
        col2node[w * WIN:w * WIN + c] = node_lo + s + np.arange(c)

    valid = col2node >= 0
    xT = np.zeros((NODE_F, nodes_pad), dtype=BF16)
    xT[:, valid] = x_bf[col2node[valid]].T

    deg_full = np.bincount(local_node, minlength=npc_k)
    deg = np.zeros((1, nodes_pad), dtype=BF16)
    deg[0, valid] = deg_full[col2node[valid] - node_lo].astype(BF16)

    return dict(xrT=xrT, eaT=eaT, colloc=colloc, degT=deg, xT=xT), col2node


# ---------------------------------------------------------------------------
# Bass program
# ---------------------------------------------------------------------------

def _build_bass(nw, tpw, relu2_split=2, b1_const=None):
    """Build the SPMD Bass program. Returns nc."""
    t_tiles = nw * tpw
    t4 = -(-t_tiles // 8) * 8
    epad = t4 * 128
    nodes_pad = nw * WIN
    ngrp = t4 // 4
    npair = ngrp // 2
    assert npair % 2 == 0 and t_tiles == t4

    dt = mybir.dt
    nc = bacc.Bacc("TRN2", target_bir_lowering=False, debug=False)

    # --- I/O ---
    xrT_d = nc.dram_tensor("xrT", [npair // 2, 128, 2048], dt.bfloat16,
                           kind="ExternalInput")
    eaT_d = nc.dram_tensor("eaT", [npair // 2, 64, 2048], dt.bfloat16,
                           kind="ExternalInput")
    colloc_d = nc.dram_tensor("colloc", [128, t4], dt.bfloat16,
                              kind="ExternalInput")
    xT_d = nc.dram_tensor("xT", [128, nodes_pad], dt.bfloat16,
                          kind="ExternalInput")
    degT_d = nc.dram_tensor("degT", [1, nodes_pad], dt.bfloat16,
                            kind="ExternalInput")
    wnames = ["W0x", "W1", "MW", "V0x", "V1", "V2"]
    w_d = {n: nc.dram_tensor(n, [128, 128], dt.bfloat16, kind="ExternalInput")
           for n in wnames}
    w0e_d = nc.dram_tensor("W0e", [64, 128], dt.bfloat16, kind="ExternalInput")
    b0_d = nc.dram_tensor("b0f", [128, 1], dt.float32, kind="ExternalInput")
    b1rep_d = nc.dram_tensor("b1rep", [1, 512], dt.bfloat16,
                             kind="ExternalInput")
    cvec_d = nc.dram_tensor("cvec", [1, 128], dt.bfloat16,
                            kind="ExternalInput")
    c0_d = nc.dram_tensor("c0f", [128, 1], dt.float32, kind="ExternalInput")
    c1_d = nc.dram_tensor("c1f", [128, 1], dt.float32, kind="ExternalInput")
    c2_d = nc.dram_tensor("c2f", [128, 1], dt.float32, kind="ExternalInput")
    b1c_d = nc.dram_tensor("b1cf", [128, 1], dt.float32, kind="ExternalInput")
    ones_d = nc.dram_tensor("ones1", [1, 128], dt.bfloat16,
                            kind="ExternalInput")
    iota_d = nc.dram_tensor("iota128", [128, 128], dt.bfloat16,
                            kind="ExternalInput")
    outT_d = nc.dram_tensor("outT", [128, nodes_pad], dt.bfloat16,
                            kind="ExternalOutput")

    # phase-B chunking: 512-node chunks; chunk c covers windows 4c..4c+3,
    # complete after pair 8(c+1)-1
    chunks = []
    c = 0
    while c < nodes_pad:
        n = min(512, nodes_pad - c)
        chunks.append((c, n))
        c += n

    with tile.TileContext(nc) as tc:
        with (
            tc.tile_pool(name="const", bufs=1) as cpool,
            tc.tile_pool(name="xr", bufs=10) as xr_pool,
            tc.tile_pool(name="ea", bufs=10) as ea_pool,
            tc.tile_pool(name="h1", bufs=4) as h1_pool,
            tc.tile_pool(name="h2n", bufs=4) as h2n_pool,
            tc.tile_pool(name="seg", bufs=3) as seg_pool,
            tc.tile_pool(name="obuf", bufs=3) as o_pool,
            tc.tile_pool(name="ph1", bufs=2, space="PSUM") as ph1_pool,
            tc.tile_pool(name="ph2", bufs=2, space="PSUM") as ph2_pool,
            tc.tile_pool(name="pu", bufs=2, space="PSUM") as pu_pool,
        ):
            def load_const(dram, shape, dtype, cname, eng=None):
                t = cpool.tile(shape, dtype, name=cname, tag=cname)
                (eng or nc.sync).dma_start(out=t[:], in_=dram.ap())
                return t

            # --- phase A constants (front of the sync DMA queue) ---
            colloc_t = load_const(colloc_d, [128, t4], dt.bfloat16, "c_colloc")
            w_t = {}
            w_t["W0x"] = load_const(w_d["W0x"], [128, 128], dt.bfloat16,
                                    "c_W0x")
            w0e_t = load_const(w0e_d, [64, 128], dt.bfloat16, "c_W0e")
            w_t["W1"] = load_const(w_d["W1"], [128, 128], dt.bfloat16, "c_W1")
            iota_t = load_const(iota_d, [128, 128], dt.bfloat16, "c_iota")
            b0_t = load_const(b0_d, [128, 1], dt.float32, "c_b0")
            b1c_t = load_const(b1c_d, [128, 1], dt.float32, "c_b1c")
            if b1_const is None:
                b1rep_t = load_const(b1rep_d, [1, 512], dt.bfloat16, "c_b1")
                ones_t = load_const(ones_d, [1, 128], dt.bfloat16, "c_ones")

            uT_t = cpool.tile([128, nodes_pad], dt.bfloat16, name="uT",
                              tag="uT")
            g1T_t = cpool.tile([128, nodes_pad], dt.bfloat16, name="g1T",
                               tag="g1T")
            g2T_t = cpool.tile([128, nodes_pad], dt.bfloat16, name="g2T",
                               tag="g2T")

            # --- PE warm-up: dense matmul burst so the HAM clock gate opens
            # (K=8/8) before phase A; runs during the DMA preamble. ---
            warm_a = ph1_pool.tile([128, 512], dt.float32, name="warm_a",
                                   tag="ph1")
            warm_b = ph1_pool.tile([128, 512], dt.float32, name="warm_b",
                                   tag="ph1")
            warm_sb = cpool.tile([128, 4], dt.bfloat16, name="warm_sb",
                                 tag="warm_sb")
            for i in range(10):
                ps = warm_a if i % 2 == 0 else warm_b
                nc.tensor.matmul(out=ps[:], lhsT=colloc_t[:, :128],
                                 rhs=colloc_t[:, :512], start=True, stop=True)
            nc.vector.tensor_copy(out=warm_sb[:], in_=warm_a[:, :4])
            nc.sync.dma_start(out=outT_d.ap()[:, 0:4], in_=warm_sb[:])

            state = {}
            iota_b = iota_t[:].rearrange("p (a b) -> p a b", a=1)

            # phase-B stage emitters (interleaved into phase A) -------------
            def stage1(ci):
                c, n = chunks[ci]
                sl = slice(c, c + n)
                pg1 = ph2_pool.tile([128, 512], dt.float32, name="pbg1",
                                    tag="ph2")
                nc.tensor.matmul(out=pg1[:, :n], lhsT=w_t["MW"][:],
                                 rhs=uT_t[:, sl], start=True, stop=False)
                nc.tensor.matmul(out=pg1[:, :n], lhsT=w_t["V0x"][:],
                                 rhs=state["xT"][:, sl], start=False,
                                 stop=False)
                nc.tensor.matmul(out=pg1[:, :n], lhsT=state["cvec"][:],
                                 rhs=state["degT"][:, sl], start=False,
                                 stop=True)
                nc.scalar.activation(g1T_t[:, sl], pg1[:, :n],
                                     mybir.ActivationFunctionType.Relu,
                                     bias=state["c0"][:])

            def stage2(ci):
                c, n = chunks[ci]
                sl = slice(c, c + n)
                pg2 = ph2_pool.tile([128, 512], dt.float32, name="pbg2",
                                    tag="ph2")
                nc.tensor.matmul(out=pg2[:, :n], lhsT=w_t["V1"][:],
                                 rhs=g1T_t[:, sl], start=True, stop=True)
                nc.scalar.activation(g2T_t[:, sl], pg2[:, :n],
                                     mybir.ActivationFunctionType.Relu,
                                     bias=state["c1"][:])

            def stage3(ci):
                c, n = chunks[ci]
                sl = slice(c, c + n)
                pg3 = ph2_pool.tile([128, 512], dt.float32, name="pbg3",
                                    tag="ph2")
                nc.tensor.matmul(out=pg3[:, :n], lhsT=w_t["V2"][:],
                                 rhs=g2T_t[:, sl], start=True, stop=True)
                ob = o_pool.tile([128, 512], dt.bfloat16, tag="ob")
                nc.scalar.activation(ob[:, :n], pg3[:, :n],
                                     mybir.ActivationFunctionType.Identity,
                                     bias=state["c2"][:])
                nc.sync.dma_start(out=outT_d.ap()[:, sl], in_=ob[:, :n])

            # ---------------- Phase A: edges (in pairs) ----------------
            pu_tiles = {}
            seg4 = None
            for p in range(npair):
                if p == 10:
                    # phase-B constants, split across both DGE queues,
                    # injected after the ramp so they don't starve it
                    state["xT"] = load_const(xT_d, [128, nodes_pad],
                                             dt.bfloat16, "c_xT",
                                             eng=nc.sync)
                    state["degT"] = load_const(degT_d, [1, nodes_pad],
                                               dt.bfloat16, "c_degT",
                                               eng=nc.sync)
                    for n in ("MW", "V0x", "V1", "V2"):
                        w_t[n] = load_const(w_d[n], [128, 128], dt.bfloat16,
                                            f"c_{n}", eng=nc.sync)
                    state["cvec"] = load_const(cvec_d, [1, 128], dt.bfloat16,
                                               "c_cvec", eng=nc.sync)
                    state["c0"] = load_const(c0_d, [128, 1], dt.float32,
                                             "c_c0", eng=nc.sync)
                    state["c1"] = load_const(c1_d, [128, 1], dt.float32,
                                             "c_c1", eng=nc.sync)
                    state["c2"] = load_const(c2_d, [128, 1], dt.float32,
                                             "c_c2", eng=nc.sync)

                # interleaved phase-B stages once a chunk's windows are done
                if p >= 24:
                    ci, ph = p // 8 - 3, p % 8
                    if ph == 0:
                        stage1(ci)
                    elif ph == 2:
                        stage2(ci)
                    elif ph == 4:
                        stage3(ci)

                if p % 2 == 0:
                    xr2 = xr_pool.tile([128, 2048], dt.bfloat16, tag="xr")
                    nc.sync.dma_start(out=xr2[:], in_=xrT_d.ap()[p // 2])
                    ea2 = ea_pool.tile([64, 2048], dt.bfloat16, tag="ea")
                    nc.sync.dma_start(out=ea2[:], in_=eaT_d.ap()[p // 2])
                xb = (p % 2) * 1024

                # two N=512 halves into one 2-bank psum tile (each MM
                # stays within a single bank); activations split ACT/DVE
                ph1 = ph1_pool.tile([128, 1024], dt.float32, tag="ph1")
                nc.tensor.matmul(out=ph1[:, :512], lhsT=w_t["W0x"][:],
                                 rhs=xr2[:, xb:xb + 512], start=True,
                                 stop=False)
                nc.tensor.matmul(out=ph1[:, 512:], lhsT=w_t["W0x"][:],
                                 rhs=xr2[:, xb + 512:xb + 1024], start=True,
                                 stop=False)
                nc.tensor.matmul(out=ph1[:, :512], lhsT=w0e_t[:],
                                 rhs=ea2[:, xb:xb + 512], start=False,
                                 stop=True)
                nc.tensor.matmul(out=ph1[:, 512:], lhsT=w0e_t[:],
                                 rhs=ea2[:, xb + 512:xb + 1024], start=False,
                                 stop=True)
                h1p = h1_pool.tile([128, 1024], dt.bfloat16, name="h1p",
                                   tag="h1")
                nc.scalar.activation(h1p[:, :512], ph1[:, :512],
                                     mybir.ActivationFunctionType.Relu,
                                     bias=b0_t[:])
                nc.vector.tensor_scalar(h1p[:, 512:], ph1[:, 512:],
                                        b0_t[:], 0.0,
                                        mybir.AluOpType.add,
                                        mybir.AluOpType.max)
                h1s = [h1p[:, :512], h1p[:, 512:]]

                for half in range(2):
                    g = 2 * p + half
                    h1 = h1s[half]
                    ph2 = ph2_pool.tile([128, 512], dt.float32, tag="ph2")
                    if b1_const is None:
                        nc.tensor.matmul(out=ph2[:], lhsT=ones_t[:],
                                         rhs=b1rep_t[:], start=True,
                                         stop=False)
                    for i in range(4):
                        sl = slice(i * 128, (i + 1) * 128)
                        nc.tensor.matmul(out=ph2[:, sl],
                                         lhsT=h1[:, sl],
                                         rhs=w_t["W1"][:],
                                         start=(b1_const is not None
                                                and i == 0),
                                         stop=(i == 3))
                    h2n = h2n_pool.tile([128, 512], dt.bfloat16, tag="h2n")
                    bb = 0.0 if b1_const is None else b1_const
                    if g % 2 == 0:
                        nc.vector.tensor_scalar(h2n[:], ph2[:], bb, 0.0,
                                                mybir.AluOpType.add,
                                                mybir.AluOpType.max)
                    else:
                        nc.scalar.activation(
                            h2n[:], ph2[:],
                            mybir.ActivationFunctionType.Relu,
                            bias=b1c_t[:] if b1_const is not None else 0.0)

                    if g % 4 == 0:
                        nbg = min(4, ngrp - g)
                        seg4 = seg_pool.tile([128, nbg * 512], dt.bfloat16,
                                             name="seg4", tag="seg",
                                             padded_shape=[128, 2048])
                        clb = colloc_t[:, g * 4:(g + nbg) * 4].to_broadcast(
                            [128, nbg * 4, 128])
                        nc.vector.tensor_tensor(
                            out=seg4[:].rearrange("p (a b) -> p a b", b=128),
                            in0=clb,
                            in1=iota_b.to_broadcast([128, nbg * 4, 128]),
                            op=mybir.AluOpType.is_equal)
                    seg = seg4[:, (g % 4) * 512:(g % 4) * 512 + 512]

                    for i in range(4):
                        t = g * 4 + i
                        w = t // tpw
                        tt = t % tpw
                        sl = slice(i * 128, (i + 1) * 128)
                        if tt == 0:
                            pu_tiles[w] = pu_pool.tile([128, 128], dt.float32,
                                                       name=f"pu{w}",
                                                       tag="pu")
                        nc.tensor.matmul(out=pu_tiles[w][:], lhsT=h2n[:, sl],
                                         rhs=seg[:, sl],
                                         start=(tt == 0), stop=(tt == tpw - 1))
                        if tt == tpw - 1:
                            nc.vector.tensor_copy(
                                out=uT_t[:, w * 128:(w + 1) * 128],
                                in_=pu_tiles[w][:])
                            del pu_tiles[w]

            # ---------------- Phase B tail ----------------
            # emit whatever stages were not emitted in-loop
            nch = len(chunks)
            s1 = {ci for ci in range(nch) if 8 * (ci + 3) < npair}
            s2 = {ci for ci in range(nch) if 8 * (ci + 3) + 2 < npair}
            s3 = {ci for ci in range(nch) if 8 * (ci + 3) + 4 < npair}
            for ci in range(nch):
                if ci not in s1:
                    stage1(ci)
            for ci in range(nch):
                if ci not in s2:
                    stage2(ci)
            for ci in range(nch):
                if ci not in s3:
                    stage3(ci)

    nc.compile()
    return nc


# ---------------------------------------------------------------------------
# Shared-weight input prep
# ---------------------------------------------------------------------------

def _prep_weights(W0, b0, W1, b1, W2, b2, V0, c0, V1, c1, V2, c2):
    W0 = _f32(W0); V0 = _f32(V0)
    V0a = V0[NODE_F:]
    MW = _f32(W2) @ V0a                     # (W2 V0a), lhsT -> V0a^T W2^T
    cvec = V0a.T @ _f32(b2).reshape(-1)     # V0a^T b2
    w = dict(
        W0x=_bf(W0[:NODE_F]),
        W0e=_bf(W0[NODE_F:]),
        W1=_bf(W1), MW=_bf(MW),
        V0x=_bf(V0[:NODE_F]),
        V1=_bf(V1), V2=_bf(V2),
        b0f=_f32(b0).reshape(128, 1),
        b1rep=_bf(np.tile(_f32(b1), 4)).reshape(1, 512),
        cvec=_bf(cvec).reshape(1, 128),
        c0f=_f32(c0).reshape(128, 1),
        c1f=_f32(c1).reshape(128, 1),
        c2f=_f32(c2).reshape(128, 1),
        b1cf=np.full((128, 1), float(np.asarray(b1).ravel()[0]), np.float32),
        ones1=np.ones((1, 128), dtype=BF16),
        iota128=np.ascontiguousarray(
            np.broadcast_to(np.arange(128), (128, 128))).astype(BF16),
    )
    return w


# ---------------------------------------------------------------------------
# Entry point
# ---------------------------------------------------------------------------

_LAST_RESULTS = {}


def kernel(x, edge_index, edge_attr, u, batch,
           W0, b0, W1, b1, W2, b2, V0, c0, V1, c1, V2, c2):
    tpw = 16

    x_bf = _bf(x)
    ea_f = _f32(edge_attr)
    row = np.asarray(edge_index[0], dtype=np.int64)
    col = np.asarray(edge_index[1], dtype=np.int64)

    order = np.argsort(col, kind="stable")
    row_s, col_s = row[order], col[order]
    ea_s = _bf(ea_f[order])

    deg_all = np.bincount(col, minlength=N_NODES)
    node_lo = [k * NPC for k in range(NCORES)]
    wins_all = [_plan_windows(deg_all[lo:lo + NPC], tpw * 128)
                for lo in node_lo]
    nw = max(len(w) for w in wins_all)

    wts = _prep_weights(W0, b0, W1, b1, W2, b2, V0, c0, V1, c1, V2, c2)

    in_maps = []
    col2node = []
    for k in range(NCORES):
        lo = node_lo[k]
        a = np.searchsorted(col_s, lo)
        b = np.searchsorted(col_s, lo + NPC)
        core, c2n = _pack_core(row_s[a:b], col_s[a:b], ea_s[a:b], x_bf, lo,
                               wins_all[k], nw, tpw)
        core.update(wts)
        in_maps.append(core)
        col2node.append(c2n)

    b1a = _f32(b1)
    b1c = float(b1a[0]) if np.all(b1a == b1a[0]) else None
    nc = _build_bass(nw, tpw, b1_const=b1c)

    trace = bool(int(os.environ.get("KERNEL_TRACE", "0")))
    kwargs = {}
    if trace:
        kwargs = dict(trace=True, trace_cores=list(range(NCORES)),
                      stitch_traces=False)
    res = run_bass_kernel_spmd(nc, in_maps, core_ids=list(range(NCORES)),
                               **kwargs)
    _LAST_RESULTS["res"] = res

    out = np.empty((N_NODES, NODE_F), dtype=np.float32)
    for k in range(NCORES):
        c2n = col2node[k]
        valid = c2n >= 0
        out[c2n[valid]] = res.results[k]["outT"][:, valid].T.astype(np.float32)
    return out


# revision 20
# speedup vs baseline: 1.0627x; 1.0627x over previous
"""GNN message-passing (NodeModel) Trainium2 kernel.

Computation (per reference):
    h   = relu(relu(concat(x[row], ea) @ W0 + b0) @ W1 + b1) @ W2 + b2   [E, 128]
    agg = segment_sum(h, col, N)                                          [N, 128]
    out = relu(relu(concat(x, agg) @ V0 + c0) @ V1 + c1) @ V2 + c2       [N, 128]

Distribution: edges are sorted by destination node; each of the 8 cores owns a
contiguous range of 6250 destination nodes and all edges pointing into it, so
no cross-core reduction is needed.  The host pre-gathers x[row] (and
transposes operands) into tile-major streaming layouts; all matmuls run in
bf16 with fp32 PSUM accumulation.

Device pipeline per core, processed in PAIRS of 512-edge groups (1024 edge
slots per pair) so L1 runs as N=1024 matmuls (one LDWEIGHTS per two groups
and 2 KB DMA descriptor rows):
  L1: h1^T = relu(W0x^T xrow^T + W0e^T ea^T + b0)      (ACT bias+relu)
  L2: h2   = relu(h1^T.T @ W1 + b1)  -- "swap" matmul per 512-group
      producing h2 in natural [edge, feat] orientation.
  Aggregation over W2 pushed past the segment sum:
      u^T[:, n] = sum_{e in seg(n)} h2[e, :]  via matmul with an on-chip
      generated one-hot segment matrix (DVE is_equal against an iota const),
      accumulated per 128-node window in PSUM.
  MLP2 with W2/V0a fused through the linear aggregation, INTERLEAVED into
  phase A as each 512-node chunk's windows complete:
      g1^T = relu(V0x^T x^T + (W2 V0a)^T u^T + (V0a^T b2) (x) deg + c0)
      out^T = V2^T relu(V1^T g1^T + c1) + c2, stored bf16.
"""

import os
import numpy as np
import ml_dtypes

import concourse.bass as bass
import concourse.bacc as bacc
import concourse.mybir as mybir
import concourse.tile as tile
from concourse.bass_utils import run_bass_kernel_spmd

BF16 = ml_dtypes.bfloat16

N_NODES = 50000
N_EDGES = 800000
NODE_F = 128
EDGE_F = 64
HID = 128
NCORES = 8
NPC = N_NODES // NCORES  # 6250 nodes per core
WIN = 128                # nodes per aggregation window (PSUM columns)


def _f32(a):
    return np.ascontiguousarray(a, dtype=np.float32)


def _bf(a):
    return np.ascontiguousarray(a, dtype=BF16)


# ---------------------------------------------------------------------------
# Host-side packing
# ---------------------------------------------------------------------------

def _plan_windows(deg_core, cap_edges, max_nodes=WIN):
    """Greedy node->window packing: consecutive nodes per window, capped at
    max_nodes nodes and cap_edges edges. Returns list of (start, n_nodes)."""
    wins = []
    s, n = 0, len(deg_core)
    while s < n:
        e = 0
        c = 0
        while s + c < n and c < max_nodes and e + deg_core[s + c] <= cap_edges:
            e += deg_core[s + c]
            c += 1
        if c == 0:
            c = 1
        wins.append((s, c))
        s += c
    return wins


def _pack_core(rows, cols, ea_sorted_bf, x_bf, node_lo, wins, nw, tpw):
    """Build per-core device input arrays.

    rows/cols: this core's edges sorted by col.
    wins: list of (node_start, n_nodes) windows for this core (len <= nw).
    Returns input dict + col->global-node map for output reassembly.
    """
    t_tiles = nw * tpw
    t4 = -(-t_tiles // 8) * 8
    epad = t4 * 128
    npair = t4 // 8
    nodes_pad = nw * WIN
    npc_k = max(w[0] + w[1] for w in wins)

    win_of_node = np.zeros(npc_k, dtype=np.int64)
    start_of_node = np.zeros(npc_k, dtype=np.int64)
    for w, (s, c) in enumerate(wins):
        win_of_node[s:s + c] = w
        start_of_node[s:s + c] = s

    local_node = cols - node_lo
    win = win_of_node[local_node]
    win_first = np.searchsorted(win, np.arange(nw))
    j = np.arange(len(cols)) - win_first[win]
    slot = win * (tpw * 128) + j
    assert j.max(initial=0) < tpw * 128

    xrowT = np.zeros((NODE_F, epad), dtype=BF16)
    xrowT[:, slot] = x_bf[rows].T
    # pair-major: each 1024-slot pair is one contiguous [128, 1024] block
    xrT = np.ascontiguousarray(
        xrowT.reshape(NODE_F, npair // 2, 2048).transpose(1, 0, 2))

    ea_full = np.zeros((EDGE_F, epad), dtype=BF16)
    ea_full[:, slot] = ea_sorted_bf.T
    eaT = np.ascontiguousarray(
        ea_full.reshape(EDGE_F, npair // 2, 2048).transpose(1, 0, 2))

    colloc = np.full((128, t4), -1.0, dtype=BF16)
    local = local_node - start_of_node[local_node]
    colloc[slot % 128, slot // 128] = local.astype(BF16)

    # column j = w*WIN + c  ->  global node node_lo + start_w + c  (c < c_w)
    col2node = np.full(nodes_pad, -1, dtype=np.int64)
    for w, (s, c) in enumerate(wins):
        col2node[w * WIN:w * WIN + c] = node_lo + s + np.arange(c)

    valid = col2node >= 0
    xT = np.zeros((NODE_F, nodes_pad), dtype=BF16)
    xT[:, valid] = x_bf[col2node[valid]].T

    deg_full = np.bincount(local_node, minlength=npc_k)
    deg = np.zeros((1, nodes_pad), dtype=BF16)
    deg[0, valid] = deg_full[col2node[valid] - node_lo].astype(BF16)

    return dict(xrT=xrT, eaT=eaT, colloc=colloc, degT=deg, xT=xT), col2node


# ---------------------------------------------------------------------------
# Bass program
# ---------------------------------------------------------------------------

def _build_bass(nw, tpw, relu2_split=2, b1_const=None):
    """Build the SPMD Bass program. Returns nc."""
    t_tiles = nw * tpw
    t4 = -(-t_tiles // 8) * 8
    epad = t4 * 128
    nodes_pad = nw * WIN
    ngrp = t4 // 4
    npair = ngrp // 2
    assert npair % 2 == 0 and t_tiles == t4

    dt = mybir.dt
    nc = bacc.Bacc("TRN2", target_bir_lowering=False, debug=False)

    # --- I/O ---
    xrT_d = nc.dram_tensor("xrT", [npair // 2, 128, 2048], dt.bfloat16,
                           kind="ExternalInput")
    eaT_d = nc.dram_tensor("eaT", [npair // 2, 64, 2048], dt.bfloat16,
                           kind="ExternalInput")
    colloc_d = nc.dram_tensor("colloc", [128, t4], dt.bfloat16,
                              kind="ExternalInput")
    xT_d = nc.dram_tensor("xT", [128, nodes_pad], dt.bfloat16,
                          kind="ExternalInput")
    degT_d = nc.dram_tensor("degT", [1, nodes_pad], dt.bfloat16,
                            kind="ExternalInput")
    wnames = ["W0x", "W1", "MW", "V0x", "V1", "V2"]
    w_d = {n: nc.dram_tensor(n, [128, 128], dt.bfloat16, kind="ExternalInput")
           for n in wnames}
    w0e_d = nc.dram_tensor("W0e", [64, 128], dt.bfloat16, kind="ExternalInput")
    b0_d = nc.dram_tensor("b0f", [128, 1], dt.float32, kind="ExternalInput")
    b1rep_d = nc.dram_tensor("b1rep", [1, 512], dt.bfloat16,
                             kind="ExternalInput")
    cvec_d = nc.dram_tensor("cvec", [1, 128], dt.bfloat16,
                            kind="ExternalInput")
    c0_d = nc.dram_tensor("c0f", [128, 1], dt.float32, kind="ExternalInput")
    c1_d = nc.dram_tensor("c1f", [128, 1], dt.float32, kind="ExternalInput")
    c2_d = nc.dram_tensor("c2f", [128, 1], dt.float32, kind="ExternalInput")
    b1c_d = nc.dram_tensor("b1cf", [128, 1], dt.float32, kind="ExternalInput")
    ones_d = nc.dram_tensor("ones1", [1, 128], dt.bfloat16,
                            kind="ExternalInput")
    iota_d = nc.dram_tensor("iota128", [128, 128], dt.bfloat16,
                            kind="ExternalInput")
    outT_d = nc.dram_tensor("outT", [128, nodes_pad], dt.bfloat16,
                            kind="ExternalOutput")

    # phase-B chunking: 512-node chunks; chunk c covers windows 4c..4c+3,
    # complete after pair 8(c+1)-1
    chunks = []
    c = 0
    while c < nodes_pad:
        n = min(512, nodes_pad - c)
        chunks.append((c, n))
        c += n

    with tile.TileContext(nc) as tc:
        with (
            tc.tile_pool(name="const", bufs=1) as cpool,
            tc.tile_pool(name="xr", bufs=10) as xr_pool,
            tc.tile_pool(name="ea", bufs=10) as ea_pool,
            tc.tile_pool(name="h1", bufs=4) as h1_pool,
            tc.tile_pool(name="h2n", bufs=4) as h2n_pool,
            tc.tile_pool(name="seg", bufs=3) as seg_pool,
            tc.tile_pool(name="obuf", bufs=3) as o_pool,
            tc.tile_pool(name="ph1", bufs=2, space="PSUM") as ph1_pool,
            tc.tile_pool(name="ph2", bufs=2, space="PSUM") as ph2_pool,
            tc.tile_pool(name="pu", bufs=2, space="PSUM") as pu_pool,
        ):
            def load_const(dram, shape, dtype, cname, eng=None):
                t = cpool.tile(shape, dtype, name=cname, tag=cname)
                (eng or nc.sync).dma_start(out=t[:], in_=dram.ap())
                return t

            # --- phase A constants (front of the sync DMA queue) ---
            colloc_t = load_const(colloc_d, [128, t4], dt.bfloat16, "c_colloc")
            w_t = {}
            w_t["W0x"] = load_const(w_d["W0x"], [128, 128], dt.bfloat16,
                                    "c_W0x")
            w0e_t = load_const(w0e_d, [64, 128], dt.bfloat16, "c_W0e")
            w_t["W1"] = load_const(w_d["W1"], [128, 128], dt.bfloat16, "c_W1")
            iota_t = load_const(iota_d, [128, 128], dt.bfloat16, "c_iota")
            b0_t = load_const(b0_d, [128, 1], dt.float32, "c_b0")
            b1c_t = load_const(b1c_d, [128, 1], dt.float32, "c_b1c")
            if b1_const is None:
                b1rep_t = load_const(b1rep_d, [1, 512], dt.bfloat16, "c_b1")
                ones_t = load_const(ones_d, [1, 128], dt.bfloat16, "c_ones")

            uT_t = cpool.tile([128, nodes_pad], dt.bfloat16, name="uT",
                              tag="uT")
            g1T_t = cpool.tile([128, nodes_pad], dt.bfloat16, name="g1T",
                               tag="g1T")
            g2T_t = cpool.tile([128, nodes_pad], dt.bfloat16, name="g2T",
                               tag="g2T")

            # --- PE warm-up: dense matmul burst so the HAM clock gate opens
            # (K=8/8) before phase A; runs during the DMA preamble. ---
            warm_a = ph1_pool.tile([128, 512], dt.float32, name="warm_a",
                                   tag="ph1")
            warm_b = ph1_pool.tile([128, 512], dt.float32, name="warm_b",
                                   tag="ph1")
            warm_sb = cpool.tile([128, 4], dt.bfloat16, name="warm_sb",
                                 tag="warm_sb")
            for i in range(10):
                ps = warm_a if i % 2 == 0 else warm_b
                nc.tensor.matmul(out=ps[:], lhsT=colloc_t[:, :128],
                                 rhs=colloc_t[:, :512], start=True, stop=True)
            nc.vector.tensor_copy(out=warm_sb[:], in_=warm_a[:, :4])
            nc.sync.dma_start(out=outT_d.ap()[:, 0:4], in_=warm_sb[:])

            state = {}
            iota_b = iota_t[:].rearrange("p (a b) -> p a b", a=1)
            iota_m = cpool.tile([128, 2048], dt.bfloat16, name="iota_m",
                                tag="iota_m")
            nc.vector.tensor_copy(
                out=iota_m[:].rearrange("p (a b) -> p a b", b=128),
                in_=iota_b.to_broadcast([128, 16, 128]))

            # phase-B stage emitters (interleaved into phase A) -------------
            def stage1(ci):
                c, n = chunks[ci]
                sl = slice(c, c + n)
                pg1 = ph2_pool.tile([128, 512], dt.float32, name="pbg1",
                                    tag="ph2")
                nc.tensor.matmul(out=pg1[:, :n], lhsT=w_t["MW"][:],
                                 rhs=uT_t[:, sl], start=True, stop=False)
                nc.tensor.matmul(out=pg1[:, :n], lhsT=w_t["V0x"][:],
                                 rhs=state["xT"][:, sl], start=False,
                                 stop=False)
                nc.tensor.matmul(out=pg1[:, :n], lhsT=state["cvec"][:],
                                 rhs=state["degT"][:, sl], start=False,
                                 stop=True)
                nc.scalar.activation(g1T_t[:, sl], pg1[:, :n],
                                     mybir.ActivationFunctionType.Relu,
                                     bias=state["c0"][:])

            def stage2(ci):
                c, n = chunks[ci]
                sl = slice(c, c + n)
                pg2 = ph2_pool.tile([128, 512], dt.float32, name="pbg2",
                                    tag="ph2")
                nc.tensor.matmul(out=pg2[:, :n], lhsT=w_t["V1"][:],
                                 rhs=g1T_t[:, sl], start=True, stop=True)
                nc.scalar.activation(g2T_t[:, sl], pg2[:, :n],
                                     mybir.ActivationFunctionType.Relu,
                                     bias=state["c1"][:])

            def stage3(ci):
                c, n = chunks[ci]
                sl = slice(c, c + n)
                pg3 = ph2_pool.tile([128, 512], dt.float32, name="pbg3",
                                    tag="ph2")
                nc.tensor.matmul(out=pg3[:, :n], lhsT=w_t["V2"][:],
                                 rhs=g2T_t[:, sl], start=True, stop=True)
                ob = o_pool.tile([128, 512], dt.bfloat16, tag="ob")
                nc.scalar.activation(ob[:, :n], pg3[:, :n],
                                     mybir.ActivationFunctionType.Identity,
                                     bias=state["c2"][:])
                nc.sync.dma_start(out=outT_d.ap()[:, sl], in_=ob[:, :n])

            # ---------------- Phase A: edges (in pairs) ----------------
            pu_tiles = {}
            seg4 = None
            for p in range(npair):
                if p == 10:
                    # phase-B constants, split across both DGE queues,
                    # injected after the ramp so they don't starve it
                    state["xT"] = load_const(xT_d, [128, nodes_pad],
                                             dt.bfloat16, "c_xT",
                                             eng=nc.sync)
                    state["degT"] = load_const(degT_d, [1, nodes_pad],
                                               dt.bfloat16, "c_degT",
                                               eng=nc.sync)
                    for n in ("MW", "V0x", "V1", "V2"):
                        w_t[n] = load_const(w_d[n], [128, 128], dt.bfloat16,
                                            f"c_{n}", eng=nc.sync)
                    state["cvec"] = load_const(cvec_d, [1, 128], dt.bfloat16,
                                               "c_cvec", eng=nc.sync)
                    state["c0"] = load_const(c0_d, [128, 1], dt.float32,
                                             "c_c0", eng=nc.sync)
                    state["c1"] = load_const(c1_d, [128, 1], dt.float32,
                                             "c_c1", eng=nc.sync)
                    state["c2"] = load_const(c2_d, [128, 1], dt.float32,
                                             "c_c2", eng=nc.sync)

                # interleaved phase-B stages once a chunk's windows are done
                if p >= 24:
                    ci, ph = p // 8 - 3, p % 8
                    if ph == 0:
                        stage1(ci)
                    elif ph == 2:
                        stage2(ci)
                    elif ph == 4:
                        stage3(ci)

                if p % 2 == 0:
                    xr2 = xr_pool.tile([128, 2048], dt.bfloat16, tag="xr")
                    nc.sync.dma_start(out=xr2[:], in_=xrT_d.ap()[p // 2])
                    ea2 = ea_pool.tile([64, 2048], dt.bfloat16, tag="ea")
                    nc.sync.dma_start(out=ea2[:], in_=eaT_d.ap()[p // 2])
                xb = (p % 2) * 1024

                # two N=512 halves into one 2-bank psum tile (each MM
                # stays within a single bank); activations split ACT/DVE
                ph1 = ph1_pool.tile([128, 1024], dt.float32, tag="ph1")
                nc.tensor.matmul(out=ph1[:, :512], lhsT=w_t["W0x"][:],
                                 rhs=xr2[:, xb:xb + 512], start=True,
                                 stop=False)
                nc.tensor.matmul(out=ph1[:, 512:], lhsT=w_t["W0x"][:],
                                 rhs=xr2[:, xb + 512:xb + 1024], start=True,
                                 stop=False)
                nc.tensor.matmul(out=ph1[:, :512], lhsT=w0e_t[:],
                                 rhs=ea2[:, xb:xb + 512], start=False,
                                 stop=True)
                nc.tensor.matmul(out=ph1[:, 512:], lhsT=w0e_t[:],
                                 rhs=ea2[:, xb + 512:xb + 1024], start=False,
                                 stop=True)
                h1p = h1_pool.tile([128, 1024], dt.bfloat16, name="h1p",
                                   tag="h1")
                nc.scalar.activation(h1p[:, :512], ph1[:, :512],
                                     mybir.ActivationFunctionType.Relu,
                                     bias=b0_t[:])
                nc.scalar.activation(h1p[:, 512:], ph1[:, 512:],
                                     mybir.ActivationFunctionType.Relu,
                                     bias=b0_t[:])
                h1s = [h1p[:, :512], h1p[:, 512:]]

                for half in range(2):
                    g = 2 * p + half
                    h1 = h1s[half]
                    ph2 = ph2_pool.tile([128, 512], dt.float32, tag="ph2")
                    if b1_const is None:
                        nc.tensor.matmul(out=ph2[:], lhsT=ones_t[:],
                                         rhs=b1rep_t[:], start=True,
                                         stop=False)
                    for i in range(4):
                        sl = slice(i * 128, (i + 1) * 128)
                        nc.tensor.matmul(out=ph2[:, sl],
                                         lhsT=h1[:, sl],
                                         rhs=w_t["W1"][:],
                                         start=(b1_const is not None
                                                and i == 0),
                                         stop=(i == 3))
                    h2n = h2n_pool.tile([128, 512], dt.bfloat16, tag="h2n")
                    bb = 0.0 if b1_const is None else b1_const
                    if b1_const is not None or g % 3 < relu2_split:
                        nc.vector.tensor_scalar(h2n[:], ph2[:], bb, 0.0,
                                                mybir.AluOpType.add,
                                                mybir.AluOpType.max)
                    else:
                        nc.scalar.activation(
                            h2n[:], ph2[:],
                            mybir.ActivationFunctionType.Relu,
                            bias=b1c_t[:] if b1_const is not None else 0.0)

                    if g % 4 == 0:
                        nbg = min(4, ngrp - g)
                        seg4 = seg_pool.tile([128, nbg * 512], dt.bfloat16,
                                             name="seg4", tag="seg",
                                             padded_shape=[128, 2048])
                        clb = colloc_t[:, g * 4:(g + nbg) * 4].to_broadcast(
                            [128, nbg * 4, 128])
                        nc.vector.tensor_tensor(
                            out=seg4[:].rearrange("p (a b) -> p a b", b=128),
                            in0=clb,
                            in1=iota_m[:, :nbg * 512].rearrange(
                                "p (a b) -> p a b", b=128),
                            op=mybir.AluOpType.is_equal)
                    seg = seg4[:, (g % 4) * 512:(g % 4) * 512 + 512]

                    for i in range(4):
                        t = g * 4 + i
                        w = t // tpw
                        tt = t % tpw
                        sl = slice(i * 128, (i + 1) * 128)
                        if tt == 0:
                            pu_tiles[w] = pu_pool.tile([128, 128], dt.float32,
                                                       name=f"pu{w}",
                                                       tag="pu")
                        nc.tensor.matmul(out=pu_tiles[w][:], lhsT=h2n[:, sl],
                                         rhs=seg[:, sl],
                                         start=(tt == 0), stop=(tt == tpw - 1))
                        if tt == tpw - 1:
                            nc.vector.tensor_copy(
                                out=uT_t[:, w * 128:(w + 1) * 128],
                                in_=pu_tiles[w][:])
                            del pu_tiles[w]

            # ---------------- Phase B tail ----------------
            # emit whatever stages were not emitted in-loop
            nch = len(chunks)
            s1 = {ci for ci in range(nch) if 8 * (ci + 3) < npair}
            s2 = {ci for ci in range(nch) if 8 * (ci + 3) + 2 < npair}
            s3 = {ci for ci in range(nch) if 8 * (ci + 3) + 4 < npair}
            for ci in range(nch):
                if ci not in s1:
                    stage1(ci)
            for ci in range(nch):
                if ci not in s2:
                    stage2(ci)
            for ci in range(nch):
                if ci not in s3:
                    stage3(ci)

    nc.compile()
    return nc


# ---------------------------------------------------------------------------
# Shared-weight input prep
# ---------------------------------------------------------------------------

def _prep_weights(W0, b0, W1, b1, W2, b2, V0, c0, V1, c1, V2, c2):
    W0 = _f32(W0); V0 = _f32(V0)
    V0a = V0[NODE_F:]
    MW = _f32(W2) @ V0a                     # (W2 V0a), lhsT -> V0a^T W2^T
    cvec = V0a.T @ _f32(b2).reshape(-1)     # V0a^T b2
    w = dict(
        W0x=_bf(W0[:NODE_F]),
        W0e=_bf(W0[NODE_F:]),
        W1=_bf(W1), MW=_bf(MW),
        V0x=_bf(V0[:NODE_F]),
        V1=_bf(V1), V2=_bf(V2),
        b0f=_f32(b0).reshape(128, 1),
        b1rep=_bf(np.tile(_f32(b1), 4)).reshape(1, 512),
        cvec=_bf(cvec).reshape(1, 128),
        c0f=_f32(c0).reshape(128, 1),
        c1f=_f32(c1).reshape(128, 1),
        c2f=_f32(c2).reshape(128, 1),
        b1cf=np.full((128, 1), float(np.asarray(b1).ravel()[0]), np.float32),
        ones1=np.ones((1, 128), dtype=BF16),
        iota128=np.ascontiguousarray(
            np.broadcast_to(np.arange(128), (128, 128))).astype(BF16),
    )
    return w


# ---------------------------------------------------------------------------
# Entry point
# ---------------------------------------------------------------------------

_LAST_RESULTS = {}


def kernel(x, edge_index, edge_attr, u, batch,
           W0, b0, W1, b1, W2, b2, V0, c0, V1, c1, V2, c2):
    tpw = 16

    x_bf = _bf(x)
    ea_f = _f32(edge_attr)
    row = np.asarray(edge_index[0], dtype=np.int64)
    col = np.asarray(edge_index[1], dtype=np.int64)

    order = np.argsort(col, kind="stable")
    row_s, col_s = row[order], col[order]
    ea_s = _bf(ea_f[order])

    deg_all = np.bincount(col, minlength=N_NODES)
    node_lo = [k * NPC for k in range(NCORES)]
    wins_all = [_plan_windows(deg_all[lo:lo + NPC], tpw * 128)
                for lo in node_lo]
    nw = max(len(w) for w in wins_all)

    wts = _prep_weights(W0, b0, W1, b1, W2, b2, V0, c0, V1, c1, V2, c2)

    in_maps = []
    col2node = []
    for k in range(NCORES):
        lo = node_lo[k]
        a = np.searchsorted(col_s, lo)
        b = np.searchsorted(col_s, lo + NPC)
        core, c2n = _pack_core(row_s[a:b], col_s[a:b], ea_s[a:b], x_bf, lo,
                               wins_all[k], nw, tpw)
        core.update(wts)
        in_maps.append(core)
        col2node.append(c2n)

    b1a = _f32(b1)
    b1c = float(b1a[0]) if np.all(b1a == b1a[0]) else None
    nc = _build_bass(nw, tpw, b1_const=b1c)

    trace = bool(int(os.environ.get("KERNEL_TRACE", "0")))
    kwargs = {}
    if trace:
        kwargs = dict(trace=True, trace_cores=list(range(NCORES)),
                      stitch_traces=False)
    res = run_bass_kernel_spmd(nc, in_maps, core_ids=list(range(NCORES)),
                               **kwargs)
    _LAST_RESULTS["res"] = res

    out = np.empty((N_NODES, NODE_F), dtype=np.float32)
    for k in range(NCORES):
        c2n = col2node[k]
        valid = c2n >= 0
        out[c2n[valid]] = res.results[k]["outT"][:, valid].T.astype(np.float32)
    return out


# revision 23
# speedup vs baseline: 1.1015x; 1.0365x over previous
"""GNN message-passing (NodeModel) Trainium2 kernel.

Computation (per reference):
    h   = relu(relu(concat(x[row], ea) @ W0 + b0) @ W1 + b1) @ W2 + b2   [E, 128]
    agg = segment_sum(h, col, N)                                          [N, 128]
    out = relu(relu(concat(x, agg) @ V0 + c0) @ V1 + c1) @ V2 + c2       [N, 128]

Distribution: edges are sorted by destination node; each of the 8 cores owns a
contiguous range of 6250 destination nodes and all edges pointing into it, so
no cross-core reduction is needed.  The host pre-gathers x[row] (and
transposes operands) into tile-major streaming layouts (one contiguous
[128,1024] block per 1024-slot pair); all matmuls run in bf16 with fp32 PSUM
accumulation.

Device pipeline per core (pairs of 512-edge groups):
  L1: h1^T = relu(W0x^T xrow^T + W0e^T ea^T + b0)      (ACT bias+relu)
  L2: h2   = relu(h1^T.T @ W1 + b1)  -- "swap" matmul per 512-group
      producing h2 in natural [edge, feat] orientation.
  Aggregation over W2 pushed past the segment sum:
      u^T[:, n] = sum_{e in seg(n)} h2[e, :]  via matmul with an on-chip
      generated one-hot segment matrix (DVE is_equal against an iota const),
      accumulated per 128-node window in PSUM.
  MLP2 with W2/V0a fused through the linear aggregation, INTERLEAVED into
  phase A as each 512-node chunk's windows complete:
      g1^T = relu(V0x^T x^T + (W2 V0a)^T u^T + (V0a^T b2) (x) deg + c0)
      out^T = V2^T relu(V1^T g1^T + c1) + c2, stored bf16.
"""

import os
import numpy as np
import ml_dtypes

import concourse.bass as bass
import concourse.bacc as bacc
import concourse.mybir as mybir
import concourse.tile as tile
from concourse.bass_utils import run_bass_kernel_spmd

BF16 = ml_dtypes.bfloat16

N_NODES = 50000
N_EDGES = 800000
NODE_F = 128
EDGE_F = 64
HID = 128
NCORES = 8
NPC = N_NODES // NCORES  # 6250 nodes per core
WIN = 128                # nodes per aggregation window (PSUM columns)


def _f32(a):
    return np.ascontiguousarray(a, dtype=np.float32)


def _bf(a):
    return np.ascontiguousarray(a, dtype=BF16)


# ---------------------------------------------------------------------------
# Host-side packing
# ---------------------------------------------------------------------------

def _plan_windows(deg_core, cap_edges, max_nodes=WIN):
    """Greedy node->window packing: consecutive nodes per window, capped at
    max_nodes nodes and cap_edges edges. Returns list of (start, n_nodes)."""
    wins = []
    s, n = 0, len(deg_core)
    while s < n:
        e = 0
        c = 0
        while s + c < n and c < max_nodes and e + deg_core[s + c] <= cap_edges:
            e += deg_core[s + c]
            c += 1
        if c == 0:
            c = 1
        wins.append((s, c))
        s += c
    return wins


def _pack_core(rows, cols, ea_sorted_bf, x_bf, node_lo, wins, nw, tpw):
    """Build per-core device input arrays.

    rows/cols: this core's edges sorted by col.
    wins: list of (node_start, n_nodes) windows for this core (len <= nw).
    Returns input dict + col->global-node map for output reassembly.
    """
    t_tiles = nw * tpw
    t4 = -(-t_tiles // 8) * 8
    epad = t4 * 128
    npair = t4 // 8
    nodes_pad = nw * WIN
    npc_k = max(w[0] + w[1] for w in wins)

    win_of_node = np.zeros(npc_k, dtype=np.int64)
    start_of_node = np.zeros(npc_k, dtype=np.int64)
    for w, (s, c) in enumerate(wins):
        win_of_node[s:s + c] = w
        start_of_node[s:s + c] = s

    local_node = cols - node_lo
    win = win_of_node[local_node]
    win_first = np.searchsorted(win, np.arange(nw))
    j = np.arange(len(cols)) - win_first[win]
    slot = win * (tpw * 128) + j
    assert j.max(initial=0) < tpw * 128

    xrowT = np.zeros((NODE_F, epad), dtype=BF16)
    xrowT[:, slot] = x_bf[rows].T
    # pair-major: each 1024-slot pair is one contiguous [128, 1024] block
    xrT = np.ascontiguousarray(
        xrowT.reshape(NODE_F, npair, 1024).transpose(1, 0, 2))

    ea_full = np.zeros((EDGE_F, epad), dtype=BF16)
    ea_full[:, slot] = ea_sorted_bf.T
    eaT = np.ascontiguousarray(
        ea_full.reshape(EDGE_F, npair, 1024).transpose(1, 0, 2))

    colloc = np.full((128, t4), -1.0, dtype=BF16)
    local = local_node - start_of_node[local_node]
    colloc[slot % 128, slot // 128] = local.astype(BF16)

    # column j = w*WIN + c  ->  global node node_lo + start_w + c  (c < c_w)
    col2node = np.full(nodes_pad, -1, dtype=np.int64)
    for w, (s, c) in enumerate(wins):
        col2node[w * WIN:w * WIN + c] = node_lo + s + np.arange(c)

    valid = col2node >= 0
    xT = np.zeros((NODE_F, nodes_pad), dtype=BF16)
    xT[:, valid] = x_bf[col2node[valid]].T

    deg_full = np.bincount(local_node, minlength=npc_k)
    deg = np.zeros((1, nodes_pad), dtype=BF16)
    deg[0, valid] = deg_full[col2node[valid] - node_lo].astype(BF16)

    return dict(xrT=xrT, eaT=eaT, colloc=colloc, degT=deg, xT=xT), col2node


# ---------------------------------------------------------------------------
# Bass program
# ---------------------------------------------------------------------------

def _build_bass(nw, tpw, relu2_split=2, b1_const=None):
    """Build the SPMD Bass program. Returns nc."""
    t_tiles = nw * tpw
    t4 = -(-t_tiles // 8) * 8
    epad = t4 * 128
    nodes_pad = nw * WIN
    ngrp = t4 // 4
    npair = ngrp // 2
    assert ngrp % 2 == 0 and t_tiles == t4

    dt = mybir.dt
    nc = bacc.Bacc("TRN2", target_bir_lowering=False, debug=False)

    # --- I/O ---
    xrT_d = nc.dram_tensor("xrT", [npair, 128, 1024], dt.bfloat16,
                           kind="ExternalInput")
    eaT_d = nc.dram_tensor("eaT", [npair, 64, 1024], dt.bfloat16,
                           kind="ExternalInput")
    colloc_d = nc.dram_tensor("colloc", [128, t4], dt.bfloat16,
                              kind="ExternalInput")
    xT_d = nc.dram_tensor("xT", [128, nodes_pad], dt.bfloat16,
                          kind="ExternalInput")
    degT_d = nc.dram_tensor("degT", [1, nodes_pad], dt.bfloat16,
                            kind="ExternalInput")
    wnames = ["W0x", "W1", "MW", "V0x", "V1", "V2"]
    w_d = {n: nc.dram_tensor(n, [128, 128], dt.bfloat16, kind="ExternalInput")
           for n in wnames}
    w0e_d = nc.dram_tensor("W0e", [64, 128], dt.bfloat16, kind="ExternalInput")
    b0_d = nc.dram_tensor("b0f", [128, 1], dt.float32, kind="ExternalInput")
    b1rep_d = nc.dram_tensor("b1rep", [1, 512], dt.bfloat16,
                             kind="ExternalInput")
    cvec_d = nc.dram_tensor("cvec", [1, 128], dt.bfloat16,
                            kind="ExternalInput")
    c0_d = nc.dram_tensor("c0f", [128, 1], dt.float32, kind="ExternalInput")
    c1_d = nc.dram_tensor("c1f", [128, 1], dt.float32, kind="ExternalInput")
    c2_d = nc.dram_tensor("c2f", [128, 1], dt.float32, kind="ExternalInput")
    b1c_d = nc.dram_tensor("b1cf", [128, 1], dt.float32, kind="ExternalInput")
    ones_d = nc.dram_tensor("ones1", [1, 128], dt.bfloat16,
                            kind="ExternalInput")
    iota_d = nc.dram_tensor("iota128", [128, 128], dt.bfloat16,
                            kind="ExternalInput")
    outT_d = nc.dram_tensor("outT", [128, nodes_pad], dt.bfloat16,
                            kind="ExternalOutput")

    # phase-B chunking: 512-node chunks; chunk c covers windows 4c..4c+3,
    # complete after pair 8(c+1)-1
    chunks = []
    c = 0
    while c < nodes_pad:
        n = min(512, nodes_pad - c)
        chunks.append((c, n))
        c += n

    with tile.TileContext(nc) as tc:
        with (
            tc.tile_pool(name="const", bufs=1) as cpool,
            tc.tile_pool(name="xr", bufs=20) as xr_pool,
            tc.tile_pool(name="ea", bufs=20) as ea_pool,
            tc.tile_pool(name="h1", bufs=4) as h1_pool,
            tc.tile_pool(name="h2n", bufs=4) as h2n_pool,
            tc.tile_pool(name="seg", bufs=3) as seg_pool,
            tc.tile_pool(name="obuf", bufs=3) as o_pool,
            tc.tile_pool(name="ph1", bufs=4, space="PSUM") as ph1_pool,
            tc.tile_pool(name="ph2", bufs=2, space="PSUM") as ph2_pool,
            tc.tile_pool(name="pu", bufs=2, space="PSUM") as pu_pool,
        ):
            def load_const(dram, shape, dtype, cname, eng=None):
                t = cpool.tile(shape, dtype, name=cname, tag=cname)
                (eng or nc.sync).dma_start(out=t[:], in_=dram.ap())
                return t

            # --- phase A constants (front of the sync DMA queue) ---
            colloc_t = load_const(colloc_d, [128, t4], dt.bfloat16, "c_colloc")
            w_t = {}
            w_t["W0x"] = load_const(w_d["W0x"], [128, 128], dt.bfloat16,
                                    "c_W0x")
            w0e_t = load_const(w0e_d, [64, 128], dt.bfloat16, "c_W0e")
            w_t["W1"] = load_const(w_d["W1"], [128, 128], dt.bfloat16, "c_W1")
            iota_t = load_const(iota_d, [128, 128], dt.bfloat16, "c_iota")
            b0_t = load_const(b0_d, [128, 1], dt.float32, "c_b0")
            b1c_t = load_const(b1c_d, [128, 1], dt.float32, "c_b1c")
            if b1_const is None:
                b1rep_t = load_const(b1rep_d, [1, 512], dt.bfloat16, "c_b1")
                ones_t = load_const(ones_d, [1, 128], dt.bfloat16, "c_ones")

            uT_t = cpool.tile([128, nodes_pad], dt.bfloat16, name="uT",
                              tag="uT")
            g1T_t = cpool.tile([128, nodes_pad], dt.bfloat16, name="g1T",
                               tag="g1T")
            g2T_t = cpool.tile([128, nodes_pad], dt.bfloat16, name="g2T",
                               tag="g2T")

            # --- PE warm-up: dense matmul burst so the HAM clock gate opens
            # (K=8/8) before phase A; runs during the DMA preamble. ---
            warm_a = ph1_pool.tile([128, 512], dt.float32, name="warm_a",
                                   tag="ph1")
            warm_b = ph1_pool.tile([128, 512], dt.float32, name="warm_b",
                                   tag="ph1")
            warm_sb = cpool.tile([128, 4], dt.bfloat16, name="warm_sb",
                                 tag="warm_sb")
            for i in range(10):
                ps = warm_a if i % 2 == 0 else warm_b
                nc.tensor.matmul(out=ps[:], lhsT=colloc_t[:, :128],
                                 rhs=colloc_t[:, :512], start=True, stop=True)
            nc.vector.tensor_copy(out=warm_sb[:], in_=warm_a[:, :4])
            nc.scalar.dma_start(out=outT_d.ap()[:, 0:4], in_=warm_sb[:])

            state = {}
            iota_b = iota_t[:].rearrange("p (a b) -> p a b", a=1)
            iota_m = cpool.tile([128, 2048], dt.bfloat16, name="iota_m",
                                tag="iota_m")
            nc.vector.tensor_copy(
                out=iota_m[:].rearrange("p (a b) -> p a b", b=128),
                in_=iota_b.to_broadcast([128, 16, 128]))

            # phase-B stage emitters (interleaved into phase A) -------------
            def stage1(ci):
                c, n = chunks[ci]
                sl = slice(c, c + n)
                pg1 = ph2_pool.tile([128, 512], dt.float32, name="pbg1",
                                    tag="ph2")
                nc.tensor.matmul(out=pg1[:, :n], lhsT=w_t["MW"][:],
                                 rhs=uT_t[:, sl], start=True, stop=False)
                nc.tensor.matmul(out=pg1[:, :n], lhsT=w_t["V0x"][:],
                                 rhs=state["xT"][:, sl], start=False,
                                 stop=False)
                nc.tensor.matmul(out=pg1[:, :n], lhsT=state["cvec"][:],
                                 rhs=state["degT"][:, sl], start=False,
                                 stop=True)
                nc.scalar.activation(g1T_t[:, sl], pg1[:, :n],
                                     mybir.ActivationFunctionType.Relu,
                                     bias=state["c0"][:])

            def stage2(ci):
                c, n = chunks[ci]
                sl = slice(c, c + n)
                pg2 = ph2_pool.tile([128, 512], dt.float32, name="pbg2",
                                    tag="ph2")
                nc.tensor.matmul(out=pg2[:, :n], lhsT=w_t["V1"][:],
                                 rhs=g1T_t[:, sl], start=True, stop=True)
                nc.scalar.activation(g2T_t[:, sl], pg2[:, :n],
                                     mybir.ActivationFunctionType.Relu,
                                     bias=state["c1"][:])

            def stage3(ci):
                c, n = chunks[ci]
                sl = slice(c, c + n)
                pg3 = ph2_pool.tile([128, 512], dt.float32, name="pbg3",
                                    tag="ph2")
                nc.tensor.matmul(out=pg3[:, :n], lhsT=w_t["V2"][:],
                                 rhs=g2T_t[:, sl], start=True, stop=True)
                ob = o_pool.tile([128, 512], dt.bfloat16, tag="ob")
                nc.scalar.activation(ob[:, :n], pg3[:, :n],
                                     mybir.ActivationFunctionType.Identity,
                                     bias=state["c2"][:])
                nc.scalar.dma_start(out=outT_d.ap()[:, sl], in_=ob[:, :n])

            # ---------------- Phase A: edges (in pairs) ----------------
            pu_tiles = {}
            seg4 = None
            for p in range(npair):
                if p == 10:
                    # phase-B constants on the scalar-engine DGE queue so
                    # they don't stall the edge stream on the sync queue
                    state["xT"] = load_const(xT_d, [128, nodes_pad],
                                             dt.bfloat16, "c_xT",
                                             eng=nc.scalar)
                    state["degT"] = load_const(degT_d, [1, nodes_pad],
                                               dt.bfloat16, "c_degT",
                                               eng=nc.scalar)
                    for n in ("MW", "V0x", "V1", "V2"):
                        w_t[n] = load_const(w_d[n], [128, 128], dt.bfloat16,
                                            f"c_{n}", eng=nc.scalar)
                    state["cvec"] = load_const(cvec_d, [1, 128], dt.bfloat16,
                                               "c_cvec", eng=nc.scalar)
                    state["c0"] = load_const(c0_d, [128, 1], dt.float32,
                                             "c_c0", eng=nc.scalar)
                    state["c1"] = load_const(c1_d, [128, 1], dt.float32,
                                             "c_c1", eng=nc.scalar)
                    state["c2"] = load_const(c2_d, [128, 1], dt.float32,
                                             "c_c2", eng=nc.scalar)

                # interleaved phase-B stages once a chunk's windows are done
                if p >= 24:
                    ci, ph = p // 8 - 3, p % 8
                    if ph == 0:
                        stage1(ci)
                    elif ph == 2:
                        stage2(ci)
                    elif ph == 4:
                        stage3(ci)

                xr = xr_pool.tile([128, 1024], dt.bfloat16, tag="xr")
                nc.sync.dma_start(out=xr[:], in_=xrT_d.ap()[p])
                ea = ea_pool.tile([64, 1024], dt.bfloat16, tag="ea")
                nc.sync.dma_start(out=ea[:], in_=eaT_d.ap()[p])

                # two N=512 halves; same-stationary matmuls adjacent
                ph1a = ph1_pool.tile([128, 512], dt.float32, tag="ph1")
                ph1b = ph1_pool.tile([128, 512], dt.float32, tag="ph1")
                nc.tensor.matmul(out=ph1a[:], lhsT=w_t["W0x"][:],
                                 rhs=xr[:, :512], start=True, stop=False)
                nc.tensor.matmul(out=ph1b[:], lhsT=w_t["W0x"][:],
                                 rhs=xr[:, 512:], start=True, stop=False)
                nc.tensor.matmul(out=ph1a[:], lhsT=w0e_t[:],
                                 rhs=ea[:, :512], start=False, stop=True)
                nc.tensor.matmul(out=ph1b[:], lhsT=w0e_t[:],
                                 rhs=ea[:, 512:], start=False, stop=True)
                h1s = [h1_pool.tile([128, 512], dt.bfloat16, name="h1a",
                                    tag="h1"),
                       h1_pool.tile([128, 512], dt.bfloat16, name="h1b",
                                    tag="h1")]
                nc.scalar.activation(h1s[0][:], ph1a[:],
                                     mybir.ActivationFunctionType.Relu,
                                     bias=b0_t[:])
                nc.scalar.activation(h1s[1][:], ph1b[:],
                                     mybir.ActivationFunctionType.Relu,
                                     bias=b0_t[:])

                for half in range(2):
                    g = 2 * p + half
                    h1 = h1s[half]
                    ph2 = ph2_pool.tile([128, 512], dt.float32, tag="ph2")
                    if b1_const is None:
                        nc.tensor.matmul(out=ph2[:], lhsT=ones_t[:],
                                         rhs=b1rep_t[:], start=True,
                                         stop=False)
                    for i in range(4):
                        sl = slice(i * 128, (i + 1) * 128)
                        nc.tensor.matmul(out=ph2[:, sl],
                                         lhsT=h1[:, sl],
                                         rhs=w_t["W1"][:],
                                         start=(b1_const is not None
                                                and i == 0),
                                         stop=(i == 3))
                    h2n = h2n_pool.tile([128, 512], dt.bfloat16, tag="h2n")
                    bb = 0.0 if b1_const is None else b1_const
                    if g % 3 < relu2_split:
                        nc.vector.tensor_scalar(h2n[:], ph2[:], bb, 0.0,
                                                mybir.AluOpType.add,
                                                mybir.AluOpType.max)
                    else:
                        nc.scalar.activation(
                            h2n[:], ph2[:],
                            mybir.ActivationFunctionType.Relu,
                            bias=b1c_t[:] if b1_const is not None else 0.0)

                    if g % 4 == 0:
                        nbg = min(4, ngrp - g)
                        seg4 = seg_pool.tile([128, nbg * 512], dt.bfloat16,
                                             name="seg4", tag="seg",
                                             padded_shape=[128, 2048])
                        clb = colloc_t[:, g * 4:(g + nbg) * 4].to_broadcast(
                            [128, nbg * 4, 128])
                        nc.vector.tensor_tensor(
                            out=seg4[:].rearrange("p (a b) -> p a b", b=128),
                            in0=clb,
                            in1=iota_m[:, :nbg * 512].rearrange(
                                "p (a b) -> p a b", b=128),
                            op=mybir.AluOpType.is_equal)
                    seg = seg4[:, (g % 4) * 512:(g % 4) * 512 + 512]

                    for i in range(4):
                        t = g * 4 + i
                        w = t // tpw
                        tt = t % tpw
                        sl = slice(i * 128, (i + 1) * 128)
                        if tt == 0:
                            pu_tiles[w] = pu_pool.tile([128, 128], dt.float32,
                                                       name=f"pu{w}",
                                                       tag="pu")
                        nc.tensor.matmul(out=pu_tiles[w][:], lhsT=h2n[:, sl],
                                         rhs=seg[:, sl],
                                         start=(tt == 0), stop=(tt == tpw - 1))
                        if tt == tpw - 1:
                            nc.vector.tensor_copy(
                                out=uT_t[:, w * 128:(w + 1) * 128],
                                in_=pu_tiles[w][:])
                            del pu_tiles[w]

            # ---------------- Phase B tail ----------------
            # emit whatever stages were not emitted in-loop
            nch = len(chunks)
            s1 = {ci for ci in range(nch) if 8 * (ci + 3) < npair}
            s2 = {ci for ci in range(nch) if 8 * (ci + 3) + 2 < npair}
            s3 = {ci for ci in range(nch) if 8 * (ci + 3) + 4 < npair}
            for ci in range(nch):
                if ci not in s1:
                    stage1(ci)
            for ci in range(nch):
                if ci not in s2:
                    stage2(ci)
            for ci in range(nch):
                if ci not in s3:
                    stage3(ci)

    nc.compile()
    return nc


# ---------------------------------------------------------------------------
# Shared-weight input prep
# ---------------------------------------------------------------------------

def _prep_weights(W0, b0, W1, b1, W2, b2, V0, c0, V1, c1, V2, c2):
    W0 = _f32(W0); V0 = _f32(V0)
    V0a = V0[NODE_F:]
    MW = _f32(W2) @ V0a                     # (W2 V0a), lhsT -> V0a^T W2^T
    cvec = V0a.T @ _f32(b2).reshape(-1)     # V0a^T b2
    w = dict(
        W0x=_bf(W0[:NODE_F]),
        W0e=_bf(W0[NODE_F:]),
        W1=_bf(W1), MW=_bf(MW),
        V0x=_bf(V0[:NODE_F]),
        V1=_bf(V1), V2=_bf(V2),
        b0f=_f32(b0).reshape(128, 1),
        b1rep=_bf(np.tile(_f32(b1), 4)).reshape(1, 512),
        cvec=_bf(cvec).reshape(1, 128),
        c0f=_f32(c0).reshape(128, 1),
        c1f=_f32(c1).reshape(128, 1),
        c2f=_f32(c2).reshape(128, 1),
        b1cf=np.full((128, 1), float(np.asarray(b1).ravel()[0]), np.float32),
        ones1=np.ones((1, 128), dtype=BF16),
        iota128=np.ascontiguousarray(
            np.broadcast_to(np.arange(128), (128, 128))).astype(BF16),
    )
    return w


# ---------------------------------------------------------------------------
# Entry point
# ---------------------------------------------------------------------------

_LAST_RESULTS = {}


def kernel(x, edge_index, edge_attr, u, batch,
           W0, b0, W1, b1, W2, b2, V0, c0, V1, c1, V2, c2):
    tpw = 16

    x_bf = _bf(x)
    ea_f = _f32(edge_attr)
    row = np.asarray(edge_index[0], dtype=np.int64)
    col = np.asarray(edge_index[1], dtype=np.int64)

    order = np.argsort(col, kind="stable")
    row_s, col_s = row[order], col[order]
    ea_s = _bf(ea_f[order])

    deg_all = np.bincount(col, minlength=N_NODES)
    node_lo = [k * NPC for k in range(NCORES)]
    wins_all = [_plan_windows(deg_all[lo:lo + NPC], tpw * 128)
                for lo in node_lo]
    nw = max(len(w) for w in wins_all)

    wts = _prep_weights(W0, b0, W1, b1, W2, b2, V0, c0, V1, c1, V2, c2)

    in_maps = []
    col2node = []
    for k in range(NCORES):
        lo = node_lo[k]
        a = np.searchsorted(col_s, lo)
        b = np.searchsorted(col_s, lo + NPC)
        core, c2n = _pack_core(row_s[a:b], col_s[a:b], ea_s[a:b], x_bf, lo,
                               wins_all[k], nw, tpw)
        core.update(wts)
        in_maps.append(core)
        col2node.append(c2n)

    b1a = _f32(b1)
    b1c = float(b1a[0]) if np.all(b1a == b1a[0]) else None
    nc = _build_bass(nw, tpw, b1_const=b1c)

    trace = bool(int(os.environ.get("KERNEL_TRACE", "0")))
    kwargs = {}
    if trace:
        kwargs = dict(trace=True, trace_cores=list(range(NCORES)),
                      stitch_traces=False)
    res = run_bass_kernel_spmd(nc, in_maps, core_ids=list(range(NCORES)),
                               **kwargs)
    _LAST_RESULTS["res"] = res

    out = np.empty((N_NODES, NODE_F), dtype=np.float32)
    for k in range(NCORES):
        c2n = col2node[k]
        valid = c2n >= 0
        out[c2n[valid]] = res.results[k]["outT"][:, valid].T.astype(np.float32)
    return out


# revision 25
# speedup vs baseline: 1.1171x; 1.0142x over previous
"""GNN message-passing (NodeModel) Trainium2 kernel.

Computation (per reference):
    h   = relu(relu(concat(x[row], ea) @ W0 + b0) @ W1 + b1) @ W2 + b2   [E, 128]
    agg = segment_sum(h, col, N)                                          [N, 128]
    out = relu(relu(concat(x, agg) @ V0 + c0) @ V1 + c1) @ V2 + c2       [N, 128]

Distribution: edges are sorted by destination node; each of the 8 cores owns a
contiguous range of 6250 destination nodes and all edges pointing into it, so
no cross-core reduction is needed.  The host pre-gathers x[row] (and
transposes operands) into tile-major streaming layouts (one contiguous
[128,1024] block per 1024-slot pair); all matmuls run in bf16 with fp32 PSUM
accumulation.

Device pipeline per core (pairs of 512-edge groups):
  L1: h1^T = relu(W0x^T xrow^T + W0e^T ea^T + b0)      (ACT bias+relu)
  L2: h2   = relu(h1^T.T @ W1 + b1)  -- "swap" matmul per 512-group
      producing h2 in natural [edge, feat] orientation.
  Aggregation over W2 pushed past the segment sum:
      u^T[:, n] = sum_{e in seg(n)} h2[e, :]  via matmul with an on-chip
      generated one-hot segment matrix (DVE is_equal against an iota const),
      accumulated per 128-node window in PSUM.
  MLP2 with W2/V0a fused through the linear aggregation, INTERLEAVED into
  phase A as each 512-node chunk's windows complete:
      g1^T = relu(V0x^T x^T + (W2 V0a)^T u^T + (V0a^T b2) (x) deg + c0)
      out^T = V2^T relu(V1^T g1^T + c1) + c2, stored bf16.
"""

import os
import numpy as np
import ml_dtypes

import concourse.bass as bass
import concourse.bacc as bacc
import concourse.mybir as mybir
import concourse.tile as tile
from concourse.bass_utils import run_bass_kernel_spmd

BF16 = ml_dtypes.bfloat16

N_NODES = 50000
N_EDGES = 800000
NODE_F = 128
EDGE_F = 64
HID = 128
NCORES = 8
NPC = N_NODES // NCORES  # 6250 nodes per core
WIN = 128                # nodes per aggregation window (PSUM columns)


def _f32(a):
    return np.ascontiguousarray(a, dtype=np.float32)


def _bf(a):
    return np.ascontiguousarray(a, dtype=BF16)


# ---------------------------------------------------------------------------
# Host-side packing
# ---------------------------------------------------------------------------

def _plan_windows(deg_core, cap_edges, max_nodes=WIN):
    """Greedy node->window packing: consecutive nodes per window, capped at
    max_nodes nodes and cap_edges edges. Returns list of (start, n_nodes)."""
    wins = []
    s, n = 0, len(deg_core)
    while s < n:
        e = 0
        c = 0
        while s + c < n and c < max_nodes and e + deg_core[s + c] <= cap_edges:
            e += deg_core[s + c]
            c += 1
        if c == 0:
            c = 1
        wins.append((s, c))
        s += c
    return wins


def _pack_core(rows, cols, ea_sorted_bf, x_bf, node_lo, wins, nw, tpw):
    """Build per-core device input arrays.

    rows/cols: this core's edges sorted by col.
    wins: list of (node_start, n_nodes) windows for this core (len <= nw).
    Returns input dict + col->global-node map for output reassembly.
    """
    t_tiles = nw * tpw
    t4 = -(-t_tiles // 8) * 8
    epad = t4 * 128
    npair = t4 // 8
    nodes_pad = nw * WIN
    npc_k = max(w[0] + w[1] for w in wins)

    win_of_node = np.zeros(npc_k, dtype=np.int64)
    start_of_node = np.zeros(npc_k, dtype=np.int64)
    for w, (s, c) in enumerate(wins):
        win_of_node[s:s + c] = w
        start_of_node[s:s + c] = s

    local_node = cols - node_lo
    win = win_of_node[local_node]
    win_first = np.searchsorted(win, np.arange(nw))
    j = np.arange(len(cols)) - win_first[win]
    slot = win * (tpw * 128) + j
    assert j.max(initial=0) < tpw * 128

    xrowT = np.zeros((NODE_F, epad), dtype=BF16)
    xrowT[:, slot] = x_bf[rows].T
    # pair-major: each 1024-slot pair is one contiguous [128, 1024] block
    xrT = np.ascontiguousarray(
        xrowT.reshape(NODE_F, npair, 1024).transpose(1, 0, 2))

    ea_full = np.zeros((EDGE_F, epad), dtype=BF16)
    ea_full[:, slot] = ea_sorted_bf.T
    eaT = np.ascontiguousarray(
        ea_full.reshape(EDGE_F, npair, 1024).transpose(1, 0, 2))

    colloc = np.full((128, t4), -1.0, dtype=BF16)
    local = local_node - start_of_node[local_node]
    colloc[slot % 128, slot // 128] = local.astype(BF16)

    # column j = w*WIN + c  ->  global node node_lo + start_w + c  (c < c_w)
    col2node = np.full(nodes_pad, -1, dtype=np.int64)
    for w, (s, c) in enumerate(wins):
        col2node[w * WIN:w * WIN + c] = node_lo + s + np.arange(c)

    valid = col2node >= 0
    xT = np.zeros((NODE_F, nodes_pad), dtype=BF16)
    xT[:, valid] = x_bf[col2node[valid]].T

    deg_full = np.bincount(local_node, minlength=npc_k)
    deg = np.zeros((1, nodes_pad), dtype=BF16)
    deg[0, valid] = deg_full[col2node[valid] - node_lo].astype(BF16)

    return dict(xrT=xrT, eaT=eaT, colloc=colloc, degT=deg, xT=xT), col2node


# ---------------------------------------------------------------------------
# Bass program
# ---------------------------------------------------------------------------

def _build_bass(nw, tpw, relu2_split=2, b1_const=None):
    """Build the SPMD Bass program. Returns nc."""
    t_tiles = nw * tpw
    t4 = -(-t_tiles // 8) * 8
    epad = t4 * 128
    nodes_pad = nw * WIN
    ngrp = t4 // 4
    npair = ngrp // 2
    assert ngrp % 2 == 0 and t_tiles == t4

    dt = mybir.dt
    nc = bacc.Bacc("TRN2", target_bir_lowering=False, debug=False)

    # --- I/O ---
    xrT_d = nc.dram_tensor("xrT", [npair, 128, 1024], dt.bfloat16,
                           kind="ExternalInput")
    eaT_d = nc.dram_tensor("eaT", [npair, 64, 1024], dt.bfloat16,
                           kind="ExternalInput")
    colloc_d = nc.dram_tensor("colloc", [128, t4], dt.bfloat16,
                              kind="ExternalInput")
    xT_d = nc.dram_tensor("xT", [128, nodes_pad], dt.bfloat16,
                          kind="ExternalInput")
    degT_d = nc.dram_tensor("degT", [1, nodes_pad], dt.bfloat16,
                            kind="ExternalInput")
    wnames = ["W0x", "W1", "MW", "V0x", "V1", "V2"]
    w_d = {n: nc.dram_tensor(n, [128, 128], dt.bfloat16, kind="ExternalInput")
           for n in wnames}
    w0e_d = nc.dram_tensor("W0e", [64, 128], dt.bfloat16, kind="ExternalInput")
    b0_d = nc.dram_tensor("b0f", [128, 1], dt.float32, kind="ExternalInput")
    b1rep_d = nc.dram_tensor("b1rep", [1, 512], dt.bfloat16,
                             kind="ExternalInput")
    cvec_d = nc.dram_tensor("cvec", [1, 128], dt.bfloat16,
                            kind="ExternalInput")
    c0_d = nc.dram_tensor("c0f", [128, 1], dt.float32, kind="ExternalInput")
    c1_d = nc.dram_tensor("c1f", [128, 1], dt.float32, kind="ExternalInput")
    c2_d = nc.dram_tensor("c2f", [128, 1], dt.float32, kind="ExternalInput")
    b1c_d = nc.dram_tensor("b1cf", [128, 1], dt.float32, kind="ExternalInput")
    ones_d = nc.dram_tensor("ones1", [1, 128], dt.bfloat16,
                            kind="ExternalInput")
    iota_d = nc.dram_tensor("iota128", [128, 128], dt.bfloat16,
                            kind="ExternalInput")
    outT_d = nc.dram_tensor("outT", [128, nodes_pad], dt.bfloat16,
                            kind="ExternalOutput")

    # phase-B chunking: 512-node chunks; chunk c covers windows 4c..4c+3,
    # complete after pair 8(c+1)-1
    chunks = []
    c = 0
    while c < nodes_pad:
        n = min(512, nodes_pad - c)
        chunks.append((c, n))
        c += n

    with tile.TileContext(nc) as tc:
        with (
            tc.tile_pool(name="const", bufs=1) as cpool,
            tc.tile_pool(name="xr", bufs=20) as xr_pool,
            tc.tile_pool(name="ea", bufs=20) as ea_pool,
            tc.tile_pool(name="h1", bufs=4) as h1_pool,
            tc.tile_pool(name="h2n", bufs=4) as h2n_pool,
            tc.tile_pool(name="seg", bufs=3) as seg_pool,
            tc.tile_pool(name="obuf", bufs=3) as o_pool,
            tc.tile_pool(name="ph1", bufs=4, space="PSUM") as ph1_pool,
            tc.tile_pool(name="ph2", bufs=2, space="PSUM") as ph2_pool,
            tc.tile_pool(name="pu", bufs=2, space="PSUM") as pu_pool,
        ):
            def load_const(dram, shape, dtype, cname, eng=None):
                t = cpool.tile(shape, dtype, name=cname, tag=cname)
                (eng or nc.sync).dma_start(out=t[:], in_=dram.ap())
                return t

            # --- phase A constants (front of the sync DMA queue) ---
            colloc_t = load_const(colloc_d, [128, t4], dt.bfloat16, "c_colloc")
            w_t = {}
            w_t["W0x"] = load_const(w_d["W0x"], [128, 128], dt.bfloat16,
                                    "c_W0x")
            w0e_t = load_const(w0e_d, [64, 128], dt.bfloat16, "c_W0e")
            w_t["W1"] = load_const(w_d["W1"], [128, 128], dt.bfloat16, "c_W1")
            iota_t = load_const(iota_d, [128, 128], dt.bfloat16, "c_iota")
            b0_t = load_const(b0_d, [128, 1], dt.float32, "c_b0")
            b1c_t = load_const(b1c_d, [128, 1], dt.float32, "c_b1c")
            if b1_const is None:
                b1rep_t = load_const(b1rep_d, [1, 512], dt.bfloat16, "c_b1")
                ones_t = load_const(ones_d, [1, 128], dt.bfloat16, "c_ones")

            uT_t = cpool.tile([128, nodes_pad], dt.bfloat16, name="uT",
                              tag="uT")
            g1T_t = cpool.tile([128, nodes_pad], dt.bfloat16, name="g1T",
                               tag="g1T")
            g2T_t = cpool.tile([128, nodes_pad], dt.bfloat16, name="g2T",
                               tag="g2T")

            # --- PE warm-up: dense matmul burst so the HAM clock gate opens
            # (K=8/8) before phase A; runs during the DMA preamble. ---
            warm_a = ph1_pool.tile([128, 512], dt.float32, name="warm_a",
                                   tag="ph1")
            warm_b = ph1_pool.tile([128, 512], dt.float32, name="warm_b",
                                   tag="ph1")
            warm_sb = cpool.tile([128, 4], dt.bfloat16, name="warm_sb",
                                 tag="warm_sb")
            for i in range(10):
                ps = warm_a if i % 2 == 0 else warm_b
                nc.tensor.matmul(out=ps[:], lhsT=colloc_t[:, :128],
                                 rhs=colloc_t[:, :512], start=True, stop=True)
            nc.vector.tensor_copy(out=warm_sb[:], in_=warm_a[:, :4])
            nc.scalar.dma_start(out=outT_d.ap()[:, 0:4], in_=warm_sb[:])

            state = {}
            iota_b = iota_t[:].rearrange("p (a b) -> p a b", a=1)
            iota_m = cpool.tile([128, 2048], dt.bfloat16, name="iota_m",
                                tag="iota_m")
            nc.vector.tensor_copy(
                out=iota_m[:].rearrange("p (a b) -> p a b", b=128),
                in_=iota_b.to_broadcast([128, 16, 128]))

            # phase-B stage emitters (interleaved into phase A) -------------
            def stage1(ci):
                c, n = chunks[ci]
                sl = slice(c, c + n)
                pg1 = ph2_pool.tile([128, 512], dt.float32, name="pbg1",
                                    tag="ph2")
                nc.tensor.matmul(out=pg1[:, :n], lhsT=w_t["MW"][:],
                                 rhs=uT_t[:, sl], start=True, stop=False)
                nc.tensor.matmul(out=pg1[:, :n], lhsT=w_t["V0x"][:],
                                 rhs=state["xT"][:, sl], start=False,
                                 stop=False)
                nc.tensor.matmul(out=pg1[:, :n], lhsT=state["cvec"][:],
                                 rhs=state["degT"][:, sl], start=False,
                                 stop=True)
                nc.scalar.activation(g1T_t[:, sl], pg1[:, :n],
                                     mybir.ActivationFunctionType.Relu,
                                     bias=state["c0"][:])

            def stage2(ci):
                c, n = chunks[ci]
                sl = slice(c, c + n)
                pg2 = ph2_pool.tile([128, 512], dt.float32, name="pbg2",
                                    tag="ph2")
                nc.tensor.matmul(out=pg2[:, :n], lhsT=w_t["V1"][:],
                                 rhs=g1T_t[:, sl], start=True, stop=True)
                nc.scalar.activation(g2T_t[:, sl], pg2[:, :n],
                                     mybir.ActivationFunctionType.Relu,
                                     bias=state["c1"][:])

            def stage3(ci):
                c, n = chunks[ci]
                sl = slice(c, c + n)
                pg3 = ph2_pool.tile([128, 512], dt.float32, name="pbg3",
                                    tag="ph2")
                nc.tensor.matmul(out=pg3[:, :n], lhsT=w_t["V2"][:],
                                 rhs=g2T_t[:, sl], start=True, stop=True)
                ob = o_pool.tile([128, 512], dt.bfloat16, tag="ob")
                nc.scalar.activation(ob[:, :n], pg3[:, :n],
                                     mybir.ActivationFunctionType.Identity,
                                     bias=state["c2"][:])
                nc.scalar.dma_start(out=outT_d.ap()[:, sl], in_=ob[:, :n])

            # ---------------- Phase A: edges (in pairs) ----------------
            pu_tiles = {}
            seg4 = None
            for p in range(npair):
                if p == 10:
                    # phase-B constants on the scalar-engine DGE queue so
                    # they don't stall the edge stream on the sync queue
                    state["xT"] = load_const(xT_d, [128, nodes_pad],
                                             dt.bfloat16, "c_xT",
                                             eng=nc.scalar)
                    state["degT"] = load_const(degT_d, [1, nodes_pad],
                                               dt.bfloat16, "c_degT",
                                               eng=nc.scalar)
                    for n in ("MW", "V0x", "V1", "V2"):
                        w_t[n] = load_const(w_d[n], [128, 128], dt.bfloat16,
                                            f"c_{n}", eng=nc.scalar)
                    state["cvec"] = load_const(cvec_d, [1, 128], dt.bfloat16,
                                               "c_cvec", eng=nc.scalar)
                    state["c0"] = load_const(c0_d, [128, 1], dt.float32,
                                             "c_c0", eng=nc.scalar)
                    state["c1"] = load_const(c1_d, [128, 1], dt.float32,
                                             "c_c1", eng=nc.scalar)
                    state["c2"] = load_const(c2_d, [128, 1], dt.float32,
                                             "c_c2", eng=nc.scalar)

                # interleaved phase-B stages once a chunk's windows are done
                if p >= 10 and (p - 10) % 8 == 0:
                    stage1((p - 10) // 8)
                elif p >= 12 and (p - 12) % 8 == 0:
                    stage2((p - 12) // 8)
                elif p >= 14 and (p - 14) % 8 == 0:
                    stage3((p - 14) // 8)

                xr = xr_pool.tile([128, 1024], dt.bfloat16, tag="xr")
                nc.sync.dma_start(out=xr[:], in_=xrT_d.ap()[p])
                ea = ea_pool.tile([64, 1024], dt.bfloat16, tag="ea")
                nc.sync.dma_start(out=ea[:], in_=eaT_d.ap()[p])

                # two N=512 halves; same-stationary matmuls adjacent
                ph1a = ph1_pool.tile([128, 512], dt.float32, tag="ph1")
                ph1b = ph1_pool.tile([128, 512], dt.float32, tag="ph1")
                nc.tensor.matmul(out=ph1a[:], lhsT=w_t["W0x"][:],
                                 rhs=xr[:, :512], start=True, stop=False)
                nc.tensor.matmul(out=ph1b[:], lhsT=w_t["W0x"][:],
                                 rhs=xr[:, 512:], start=True, stop=False)
                nc.tensor.matmul(out=ph1a[:], lhsT=w0e_t[:],
                                 rhs=ea[:, :512], start=False, stop=True)
                nc.tensor.matmul(out=ph1b[:], lhsT=w0e_t[:],
                                 rhs=ea[:, 512:], start=False, stop=True)
                h1s = [h1_pool.tile([128, 512], dt.bfloat16, name="h1a",
                                    tag="h1"),
                       h1_pool.tile([128, 512], dt.bfloat16, name="h1b",
                                    tag="h1")]
                nc.scalar.activation(h1s[0][:], ph1a[:],
                                     mybir.ActivationFunctionType.Relu,
                                     bias=b0_t[:])
                nc.scalar.activation(h1s[1][:], ph1b[:],
                                     mybir.ActivationFunctionType.Relu,
                                     bias=b0_t[:])

                for half in range(2):
                    g = 2 * p + half
                    h1 = h1s[half]
                    ph2 = ph2_pool.tile([128, 512], dt.float32, tag="ph2")
                    if b1_const is None:
                        nc.tensor.matmul(out=ph2[:], lhsT=ones_t[:],
                                         rhs=b1rep_t[:], start=True,
                                         stop=False)
                    for i in range(4):
                        sl = slice(i * 128, (i + 1) * 128)
                        nc.tensor.matmul(out=ph2[:, sl],
                                         lhsT=h1[:, sl],
                                         rhs=w_t["W1"][:],
                                         start=(b1_const is not None
                                                and i == 0),
                                         stop=(i == 3))
                    h2n = h2n_pool.tile([128, 512], dt.bfloat16, tag="h2n")
                    bb = 0.0 if b1_const is None else b1_const
                    if g % 3 < relu2_split:
                        nc.vector.tensor_scalar(h2n[:], ph2[:], bb, 0.0,
                                                mybir.AluOpType.add,
                                                mybir.AluOpType.max)
                    else:
                        nc.scalar.activation(
                            h2n[:], ph2[:],
                            mybir.ActivationFunctionType.Relu,
                            bias=b1c_t[:] if b1_const is not None else 0.0)

                    if g % 4 == 0:
                        nbg = min(4, ngrp - g)
                        seg4 = seg_pool.tile([128, nbg * 512], dt.bfloat16,
                                             name="seg4", tag="seg",
                                             padded_shape=[128, 2048])
                        clb = colloc_t[:, g * 4:(g + nbg) * 4].to_broadcast(
                            [128, nbg * 4, 128])
                        nc.vector.tensor_tensor(
                            out=seg4[:].rearrange("p (a b) -> p a b", b=128),
                            in0=clb,
                            in1=iota_m[:, :nbg * 512].rearrange(
                                "p (a b) -> p a b", b=128),
                            op=mybir.AluOpType.is_equal)
                    seg = seg4[:, (g % 4) * 512:(g % 4) * 512 + 512]

                    for i in range(4):
                        t = g * 4 + i
                        w = t // tpw
                        tt = t % tpw
                        sl = slice(i * 128, (i + 1) * 128)
                        if tt == 0:
                            pu_tiles[w] = pu_pool.tile([128, 128], dt.float32,
                                                       name=f"pu{w}",
                                                       tag="pu")
                        nc.tensor.matmul(out=pu_tiles[w][:], lhsT=h2n[:, sl],
                                         rhs=seg[:, sl],
                                         start=(tt == 0), stop=(tt == tpw - 1))
                        if tt == tpw - 1:
                            nc.vector.tensor_copy(
                                out=uT_t[:, w * 128:(w + 1) * 128],
                                in_=pu_tiles[w][:])
                            del pu_tiles[w]

            # ---------------- Phase B tail ----------------
            # emit whatever stages were not emitted in-loop
            nch = len(chunks)
            s1 = {ci for ci in range(nch) if 8 * ci + 10 < npair}
            s2 = {ci for ci in range(nch) if 8 * ci + 12 < npair}
            s3 = {ci for ci in range(nch) if 8 * ci + 14 < npair}
            for ci in range(nch):
                if ci not in s1:
                    stage1(ci)
            for ci in range(nch):
                if ci not in s2:
                    stage2(ci)
            for ci in range(nch):
                if ci not in s3:
                    stage3(ci)

    nc.compile()
    return nc


# ---------------------------------------------------------------------------
# Shared-weight input prep
# ---------------------------------------------------------------------------

def _prep_weights(W0, b0, W1, b1, W2, b2, V0, c0, V1, c1, V2, c2):
    W0 = _f32(W0); V0 = _f32(V0)
    V0a = V0[NODE_F:]
    MW = _f32(W2) @ V0a                     # (W2 V0a), lhsT -> V0a^T W2^T
    cvec = V0a.T @ _f32(b2).reshape(-1)     # V0a^T b2
    w = dict(
        W0x=_bf(W0[:NODE_F]),
        W0e=_bf(W0[NODE_F:]),
        W1=_bf(W1), MW=_bf(MW),
        V0x=_bf(V0[:NODE_F]),
        V1=_bf(V1), V2=_bf(V2),
        b0f=_f32(b0).reshape(128, 1),
        b1rep=_bf(np.tile(_f32(b1), 4)).reshape(1, 512),
        cvec=_bf(cvec).reshape(1, 128),
        c0f=_f32(c0).reshape(128, 1),
        c1f=_f32(c1).reshape(128, 1),
        c2f=_f32(c2).reshape(128, 1),
        b1cf=np.full((128, 1), float(np.asarray(b1).ravel()[0]), np.float32),
        ones1=np.ones((1, 128), dtype=BF16),
        iota128=np.ascontiguousarray(
            np.broadcast_to(np.arange(128), (128, 128))).astype(BF16),
    )
    return w


# ---------------------------------------------------------------------------
# Entry point
# ---------------------------------------------------------------------------

_LAST_RESULTS = {}


def kernel(x, edge_index, edge_attr, u, batch,
           W0, b0, W1, b1, W2, b2, V0, c0, V1, c1, V2, c2):
    tpw = 16

    x_bf = _bf(x)
    ea_f = _f32(edge_attr)
    row = np.asarray(edge_index[0], dtype=np.int64)
    col = np.asarray(edge_index[1], dtype=np.int64)

    order = np.argsort(col, kind="stable")
    row_s, col_s = row[order], col[order]
    ea_s = _bf(ea_f[order])

    deg_all = np.bincount(col, minlength=N_NODES)
    node_lo = [k * NPC for k in range(NCORES)]
    wins_all = [_plan_windows(deg_all[lo:lo + NPC], tpw * 128)
                for lo in node_lo]
    nw = max(len(w) for w in wins_all)

    wts = _prep_weights(W0, b0, W1, b1, W2, b2, V0, c0, V1, c1, V2, c2)

    in_maps = []
    col2node = []
    for k in range(NCORES):
        lo = node_lo[k]
        a = np.searchsorted(col_s, lo)
        b = np.searchsorted(col_s, lo + NPC)
        core, c2n = _pack_core(row_s[a:b], col_s[a:b], ea_s[a:b], x_bf, lo,
                               wins_all[k], nw, tpw)
        core.update(wts)
        in_maps.append(core)
        col2node.append(c2n)

    b1a = _f32(b1)
    b1c = float(b1a[0]) if np.all(b1a == b1a[0]) else None
    nc = _build_bass(nw, tpw, b1_const=b1c)

    trace = bool(int(os.environ.get("KERNEL_TRACE", "0")))
    kwargs = {}
    if trace:
        kwargs = dict(trace=True, trace_cores=list(range(NCORES)),
                      stitch_traces=False)
    res = run_bass_kernel_spmd(nc, in_maps, core_ids=list(range(NCORES)),
                               **kwargs)
    _LAST_RESULTS["res"] = res

    out = np.empty((N_NODES, NODE_F), dtype=np.float32)
    for k in range(NCORES):
        c2n = col2node[k]
        valid = c2n >= 0
        out[c2n[valid]] = res.results[k]["outT"][:, valid].T.astype(np.float32)
    return out
